# revision 32
# baseline (speedup 1.0000x reference)
"""Causal single-head attention (B=4, S=4096, D=1024) on 8 TRN2 NeuronCores.

Sharding: core = (batch b, half h).  Each core computes attention output for
2048 queries of one batch: query chunks {0,3,4,7} (h=0) or {1,2,5,6} (h=1) of
8x512, which balances causal work.

Design (best measured 424,968ns max-core / 418us mean on a healthy fleet,
vs 654,126ns fp16 baseline; absmax-rel err 1.81e-2 < 2e-2 gate):
  * Q/K projections and the QK^T scores matmul run in fp8e4 with
    MatmulPerfMode.DoubleRow (2 contraction sub-tiles per instruction;
    measured issue rate ~216-260ns per 256-deep x 512-wide MM, ~2x fp16).
    W_q/W_k are pre-scaled by 32 on the host; the extra 32*32 is folded
    into the exp scale (2^-15).  V projection / P*V context matmul stay
    fp16: fp8 V fails the 2e-2 gate (absmax-rel ~5e-2 in numpy sim).
  * K^T and V projections are deduplicated across the core pair sharing a
    batch: core (b,h) projects key chunks 4h..4h+3 only; AllGather
    collectives over pairs {2b,2b+1} exchange K^T (fp8, two half-gathers
    issued as their inputs land, each followed by contiguous readbacks)
    and V (fp16, two 2MB gathers) through DRAM, overlapped with the
    remaining projection work so attention never waits on peer data.
  * K^T lives in 8 per-chunk SBUF tiles [P,8,CH] so the post-collective
    readback DMA is contiguous per partition.
  * Startup: wk is loaded as 8 per-o-block pieces and xk chunks as 4
    dp-pair pieces so the first projection matmul starts as soon as the
    first ~256KB lands; all small constants ride in ONE merged [P,768]
    DMA; a memset-fed stream of dummy matmuls warms the PE HAM clock gate
    (cold 1.2GHz -> warm 2.4GHz).
  * The attention block loop is software-pipelined (scores of block k+1
    are enqueued on the Tensor queue before den/ctx of block k) so the
    exp->mask latency never stalls the PE; each slot's final block fuses
    the epilogue, taking 1/den on the [1,512] row BEFORE broadcasting it
    through the ones-matmul, and the output mult reads the broadcast
    directly from PSUM — keeping the kernel tail short.
Scores are computed in the S^T = [k, q] layout so no on-device transposes
are needed anywhere; P = exp(s*scale) * causal_mask with the mask built
from dmat/amat per-core data (mask = (iota_k - iota_q) <= a[slot,j]).
"""

import sys

for _p in ("/opt/trn_rl_repo",):
    if _p not in sys.path:
        sys.path.insert(0, _p)

import numpy as np

B, S, D = 4, 4096, 1024
P = 128
CH = 512                       # query chunk
NSLOT = 4                      # chunks per core
NQ = NSLOT * CH                # queries per core
NK = [8, 16, 24, 32]           # k-tiles per slot (uniform across cores)
SLOTBASE = [0, 8, 24, 48]      # amat column base per slot
CHUNKS_H = [[0, 3, 4, 7], [1, 2, 5, 6]]
WSCALE = 32.0                  # host pre-scale on W_q/W_k
SCALE = 1.0 / 32.0 / (WSCALE * WSCALE)   # exp scale = 2^-15

# merged const layout: [P, CW] fp16
C_DMAT = 0      # cols 0..511: dmat
C_AMAT = 512    # cols 512..591: amat
C_ONEK = 592    # col 592: ones (ones_k)
C_ONER = 608    # row 0, cols 608..735: ones (ones_r)
CW = 768

PROJ_FP8 = True                # Q/K projections fp8 DoubleRow
SCORES_FP8 = True              # QK^T scores fp8 DoubleRow
USE_GATHER = True              # dedup K/V projection across core pairs
WARMUP_MM = 16

_PROGRAM = None


def _build_program():
    import concourse.bass as bass
    import concourse.tile as tile
    import concourse.mybir as mybir
    from concourse import bacc
    from concourse.bass import ds, ts

    f32 = mybir.dt.float32
    f16 = mybir.dt.float16
    f8 = mybir.dt.float8e4
    DR = mybir.MatmulPerfMode.DoubleRow

    pj_dt = f8 if PROJ_FP8 else f16
    sc_dt = f8 if SCORES_FP8 else f16

    nc = bacc.Bacc(trn_type="TRN2", target_bir_lowering=False, debug=False,
                   num_devices=8)

    NKC = 4 if USE_GATHER else 8   # key chunks projected locally
    # wkT: [o_blk, P, d_slab, P]; xkT: [chunk, dp_pair, P, 2, CH]
    wkT = nc.declare_dram_parameter("wkT", [8, P, 8, P], pj_dt, isOutput=False)
    xkT = nc.declare_dram_parameter("xkT", [NKC, 4, P, 2, CH], pj_dt,
                                    isOutput=False)
    xvT = nc.declare_dram_parameter("xvT", [NKC, P, 8, CH], f16, isOutput=False)
    xqT = nc.declare_dram_parameter("xqT", [NSLOT, P, 8, CH], pj_dt,
                                    isOutput=False)
    wqT = nc.declare_dram_parameter("wqT", [P, 8, D], pj_dt, isOutput=False)
    wvT = nc.declare_dram_parameter("wvT", [P, 8, D], f16, isOutput=False)
    cmat = nc.declare_dram_parameter("cmat", [P, CW], f16, isOutput=False)
    outT = nc.declare_dram_parameter("outT", [D, NQ], f32, isOutput=True)

    if USE_GATHER:
        kx_in = nc.dram_tensor("kx_in", [4, P, 8, CH], sc_dt)
        kx_outA = nc.dram_tensor("kx_outA", [2, 2, P, 8, CH], sc_dt)
        kx_outB = nc.dram_tensor("kx_outB", [2, 2, P, 8, CH], sc_dt)
        v_in = nc.dram_tensor("v_in", [4, CH, D], f16)
        v_outA = nc.dram_tensor("v_outA", [2, 2, CH, D], f16)
        v_outB = nc.dram_tensor("v_outB", [2, 2, CH, D], f16)
        RG = [[0, 1], [2, 3], [4, 5], [6, 7]]
    else:
        vscr = nc.dram_tensor("v_scratch", [S, D], f16)

    Exp = mybir.ActivationFunctionType.Exp
    is_le = mybir.AluOpType.is_le
    mult = mybir.AluOpType.mult
    bypass = mybir.AluOpType.bypass

    def v_src(j):
        """AP of V rows [j*128, j*128+128) in the gathered layout."""
        cid = j // 4
        if not USE_GATHER:
            return vscr[ds(j * P, P), :]
        r, i = cid // 4, cid % 4
        vo = v_outA if i < 2 else v_outB
        return vo[r][i % 2, ds((j % 4) * P, P), :]

    with tile.TileContext(nc, pool_alloc_mode="queue") as tc:
        with (
            tc.tile_pool(name="kt", bufs=1) as kt_pool,
            tc.tile_pool(name="qt", bufs=1) as qt_pool,
            tc.tile_pool(name="const", bufs=1) as const_pool,
        ):
            # K^T piece tiles: KTpc[cid] = key chunk cid
            KTpc = [
                kt_pool.tile([P, 8, CH], sc_dt, tag=f"kt{c}", name=f"KTpc{c}")
                for c in range(8)
            ]
            QTs = [
                qt_pool.tile([P, 8, CH], sc_dt, tag=f"qt{i}", name=f"QTs{i}")
                for i in range(NSLOT)
            ]
            cm = const_pool.tile([P, CW], f16, tag="cmat")
            wtile = const_pool.tile([P, CH], f16, tag="warm")
            dmat_sb = cm[:, ds(C_DMAT, CH)]
            amat_sb = cm[:, ds(C_AMAT, 80)]
            ones_k_sb = cm[:, ds(C_ONEK, 1)]
            ones_r_sb = cm[ds(0, 1), ds(C_ONER, P)]

            # warmup: memset-fed dummy matmuls warm the PE HAM clock gate
            nc.vector.memset(wtile[:], 1.0)

            # ---------- Phase 0+1: projections K, V, Q ---------------------
            with (
                tc.tile_pool(name="w0", bufs=1) as w_pool,
                tc.tile_pool(name="xk", bufs=4) as xk_pool,
                tc.tile_pool(name="xv", bufs=4) as xv_pool,
                tc.tile_pool(name="xq", bufs=4) as xq_pool,
                tc.tile_pool(name="kb", bufs=2) as kb_pool,
                tc.tile_pool(name="vb", bufs=6) as vb_pool,
                tc.tile_pool(name="ps0", bufs=4, space="PSUM") as ps_pool,
                tc.tile_pool(name="wps", bufs=1, space="PSUM") as wu_pool,
            ):
                # xk chunks split across sync (0,1) and scalar (2,3) so
                # kproj is never starved by a single DMA queue; dp-pair
                # pieces so the first matmul starts on the first ~128KB
                xk_tiles = []
                for c in range(4):
                    xk = xk_pool.tile([P, 8, CH], pj_dt, tag="xk",
                                      name=f"xk{c}")
                    eng = nc.sync if c < 2 else nc.scalar
                    for dp in range(4):
                        eng.dma_start(out=xk[:, ds(2 * dp, 2), :],
                                      in_=xkT[c, dp])
                    xk_tiles.append(xk)
                nc.sync.dma_start(out=cm[:], in_=cmat[:])
                # wk: 8 per-o-block pieces on gpsimd, o ascending
                wk_os = []
                for o in range(8):
                    wko = w_pool.tile([P, 8, P], pj_dt, tag=f"wk{o}",
                                      name=f"wk{o}")
                    nc.gpsimd.dma_start(out=wko[:], in_=wkT[o])
                    wk_os.append(wko)
                wv = w_pool.tile([P, 8, D], f16, tag="wv")
                wq = w_pool.tile([P, 8, D], pj_dt, tag="wq")
                nc.scalar.dma_start(out=wv[:], in_=wvT[:])

                for _ in range(WARMUP_MM):
                    wps = wu_pool.tile([1, CH], f32, name="warm")
                    nc.tensor.matmul(
                        wps[:], lhsT=wtile[:, ds(0, 1)], rhs=wtile[:],
                        start=True, stop=True,
                    )

                def kproj_mms(ps, o, xk):
                    if PROJ_FP8:
                        for dp in range(4):
                            nc.tensor.matmul(
                                ps[:],
                                lhsT=wk_os[o][:, ds(2 * dp, 2), :],
                                rhs=xk[:, ds(2 * dp, 2), :],
                                start=(dp == 0),
                                stop=(dp == 3),
                                perf_mode=DR,
                            )
                    else:
                        for d in range(8):
                            nc.tensor.matmul(
                                ps[:],
                                lhsT=wk_os[o][:, d, :],
                                rhs=xk[:, d, :],
                                start=(d == 0),
                                stop=(d == 7),
                            )

                with nc.named_scope("kproj"):
                    for c in range(NKC):
                        if c < 4:
                            xk = xk_tiles[c]
                        else:
                            xk = xk_pool.tile(
                                [P, 8, CH], pj_dt, tag="xk", name=f"xk{c}"
                            )
                            for dp in range(4):
                                nc.sync.dma_start(
                                    out=xk[:, ds(2 * dp, 2), :], in_=xkT[c, dp]
                                )
                        if USE_GATHER:
                            kb = kb_pool.tile(
                                [P, 8, CH], sc_dt, tag="kb", name=f"kb{c}"
                            )
                        for o in range(8):
                            ps = ps_pool.tile([P, CH], f32, tag="ps", name="psk")
                            kproj_mms(ps, o, xk)
                            if USE_GATHER:
                                nc.vector.tensor_copy(kb[:, o, :], ps[:])
                            else:
                                nc.vector.tensor_copy(KTpc[c][:, o, :], ps[:])
                        if USE_GATHER:
                            nc.sync.dma_start(out=kx_in[c], in_=kb[:])
                            if c == 1 or c == 3:
                                # two half-gathers: the first (chunks 0,1 +
                                # peer 4,5) completes ~30us earlier than a
                                # merged gather, unblocking attention start
                                kxo = kx_outA if c == 1 else kx_outB
                                lo = 0 if c == 1 else 2
                                nc.gpsimd.collective_compute(
                                    "AllGather", bypass, RG,
                                    ins=[kx_in[ds(lo, 2)].opt()],
                                    outs=[kxo[ds(0, 2)].opt()],
                                )
                                for r in range(2):
                                    for i in range(2):
                                        nc.gpsimd.dma_start(
                                            out=KTpc[4 * r + lo + i][:],
                                            in_=kxo[r][i],
                                        )

                # wq/xq ride the sync queue behind the compute-gated kb
                # writes: they are needed only at qproj (~105us) and this
                # keeps them out of the startup bandwidth window
                nc.sync.dma_start(out=wq[:], in_=wqT[:])
                xq_tiles = []
                for s in range(NSLOT):
                    xq = xq_pool.tile([P, 8, CH], pj_dt, tag="xq", name=f"xq{s}")
                    nc.sync.dma_start(out=xq[:], in_=xqT[s])
                    xq_tiles.append(xq)

                with nc.named_scope("vproj"):
                    for c in range(NKC):
                        xv = xv_pool.tile(
                            [P, 8, CH], f16, tag="xv", name=f"xv{c}"
                        )
                        nc.sync.dma_start(out=xv[:], in_=xvT[c])
                        for kt_i in range(4):
                            vb = vb_pool.tile([P, D], f16, tag="vb", name="vb")
                            for oh in range(2):
                                ps = ps_pool.tile(
                                    [P, CH], f32, tag="ps", name="psv"
                                )
                                for d in range(8):
                                    nc.tensor.matmul(
                                        ps[:],
                                        lhsT=xv[:, d, ts(kt_i, P)],
                                        rhs=wv[:, d, ts(oh, CH)],
                                        start=(d == 0),
                                        stop=(d == 7),
                                    )
                                # DVE is idle during vproj; the ACT-engine
                                # copy measured ~1.7us each and serialized
                                # the whole phase through the psum pool
                                nc.vector.tensor_copy(vb[:, ts(oh, CH)], ps[:])
                            if USE_GATHER:
                                # scalar queue is idle here; sync is busy
                                # with kb writes — fast drain keeps the
                                # 3-deep vb pool recycling
                                nc.scalar.dma_start(
                                    out=v_in[c][ds(kt_i * P, P), :], in_=vb[:]
                                )
                            else:
                                nc.scalar.dma_start(
                                    out=vscr[ds(c * CH + kt_i * P, P), :],
                                    in_=vb[:],
                                )
                        if USE_GATHER and c == 1:
                            nc.gpsimd.collective_compute(
                                "AllGather", bypass, RG,
                                ins=[v_in[ds(0, 2)].opt()],
                                outs=[v_outA[ds(0, 2)].opt()],
                            )
                    if USE_GATHER:
                        nc.gpsimd.collective_compute(
                            "AllGather", bypass, RG,
                            ins=[v_in[ds(2, 2)].opt()],
                            outs=[v_outB[ds(0, 2)].opt()],
                        )

                with nc.named_scope("qproj"):
                    for s in range(NSLOT):
                        for o in range(8):
                            ps = ps_pool.tile([P, CH], f32, tag="ps", name="psq")
                            if PROJ_FP8:
                                for dp in range(4):
                                    nc.tensor.matmul(
                                        ps[:],
                                        lhsT=wq[:, ds(2 * dp, 2), ts(o, P)],
                                        rhs=xq_tiles[s][:, ds(2 * dp, 2), :],
                                        start=(dp == 0),
                                        stop=(dp == 3),
                                        perf_mode=DR,
                                    )
                            else:
                                for d in range(8):
                                    nc.tensor.matmul(
                                        ps[:],
                                        lhsT=wq[:, d, ts(o, P)],
                                        rhs=xq_tiles[s][:, d, :],
                                        start=(d == 0),
                                        stop=(d == 7),
                                    )
                            nc.vector.tensor_copy(QTs[s][:, o, :], ps[:])

            # ---------------- Phase 2: attention ---------------------------
            with (
                tc.tile_pool(name="ctx", bufs=2) as ctx_pool,
                tc.tile_pool(name="vt", bufs=12) as v_pool,
                tc.tile_pool(name="pt", bufs=12) as p_pool,
                tc.tile_pool(name="et", bufs=3) as e_pool,
                tc.tile_pool(name="fo", bufs=4) as f_pool,
                tc.tile_pool(name="dsb", bufs=4) as den_pool,
                tc.tile_pool(name="pss", bufs=3, space="PSUM") as s_ps_pool,
                tc.tile_pool(name="psc", bufs=3, space="PSUM") as c_ps_pool,
                tc.tile_pool(name="psd", bufs=1, space="PSUM") as d_ps_pool,
                tc.tile_pool(name="psb", bufs=1, space="PSUM") as b_ps_pool,
                nc.named_scope("attn"),
            ):
                slot_state = {}

                def emit_scores(slot, blk):
                    """Load V tiles, compute scores -> exp -> mask for one
                    4-k-tile block.  Returns (p_tiles, v_tiles)."""
                    p_tiles, v_tiles = [], []
                    for j4 in range(4):
                        j = blk * 4 + j4
                        vt = v_pool.tile([P, D], f16, tag="vt", name="vt")
                        nc.scalar.dma_start(out=vt[:], in_=v_src(j))
                        KT = KTpc[j // 4]
                        sps = s_ps_pool.tile([P, CH], f32, name="sps")
                        if SCORES_FP8:
                            for op in range(4):
                                nc.tensor.matmul(
                                    sps[:],
                                    lhsT=KT[:, ds(2 * op, 2), ds((j % 4) * P, P)],
                                    rhs=QTs[slot][:, ds(2 * op, 2), :],
                                    start=(op == 0),
                                    stop=(op == 3),
                                    perf_mode=DR,
                                )
                        else:
                            for o in range(8):
                                nc.tensor.matmul(
                                    sps[:],
                                    lhsT=KT[:, o, ds((j % 4) * P, P)],
                                    rhs=QTs[slot][:, o, :],
                                    start=(o == 0),
                                    stop=(o == 7),
                                )
                        et = e_pool.tile([P, CH], f16, tag="et", name="et")
                        nc.scalar.activation(et[:], sps[:], Exp, scale=SCALE)
                        pt = p_pool.tile([P, CH], f16, tag="pt", name="pt")
                        col = SLOTBASE[slot] + j
                        nc.vector.scalar_tensor_tensor(
                            out=pt[:],
                            in0=dmat_sb,
                            scalar=amat_sb[:, ds(col, 1)],
                            in1=et[:],
                            op0=is_le,
                            op1=mult,
                        )
                        p_tiles.append(pt)
                        v_tiles.append(vt)
                    return p_tiles, v_tiles

                def emit_consume(slot, blk, tiles):
                    """den + ctx matmuls for a block; on the slot's final
                    block, fuse the epilogue (reciprocal, out mult + DMA)."""
                    p_tiles, v_tiles = tiles
                    final = blk == NK[slot] // 4 - 1
                    st = slot_state[slot]
                    ctx, den = st["ctx"], st["den"]
                    dps = d_ps_pool.tile([1, CH], f32, name="dps")
                    for j4 in range(4):
                        nc.tensor.matmul(
                            dps[:],
                            lhsT=ones_k_sb,
                            rhs=p_tiles[j4][:],
                            start=(j4 == 0),
                            stop=(j4 == 3),
                        )
                    if blk == 0:
                        nc.vector.tensor_copy(den[:], dps[:])
                    else:
                        nc.vector.tensor_add(den[:], den[:], dps[:])
                    if final:
                        # reciprocal on the [1,CH] row (cheap), then
                        # broadcast the reciprocal across partitions with
                        # the ones_r matmul; mults read the PSUM directly
                        rrow = den_pool.tile([1, CH], f32, tag="rrow",
                                             name="rrow")
                        nc.vector.reciprocal(rrow[:], den[:])
                        r16 = den_pool.tile([1, CH], f16, tag="r16",
                                            name="r16")
                        nc.vector.tensor_copy(r16[:], rrow[:])
                        bps = b_ps_pool.tile([P, CH], f32, name="bps")
                        nc.tensor.matmul(
                            bps[:], lhsT=ones_r_sb, rhs=r16[:],
                            start=True, stop=True,
                        )
                    for o in range(8):
                        cps = c_ps_pool.tile([P, CH], f32, name="cps")
                        for j4 in range(4):
                            nc.tensor.matmul(
                                cps[:],
                                lhsT=v_tiles[j4][:, ts(o, P)],
                                rhs=p_tiles[j4][:],
                                start=(j4 == 0),
                                stop=(j4 == 3),
                            )
                        if blk == 0:
                            nc.vector.tensor_copy(ctx[:, o, :], cps[:])
                        else:
                            nc.vector.tensor_add(
                                ctx[:, o, :], ctx[:, o, :], cps[:]
                            )
                        if final:
                            ft = f_pool.tile([P, CH], f32, tag="ft", name="ft")
                            nc.vector.tensor_mul(ft[:], ctx[:, o, :], bps[:])
                            nc.sync.dma_start(
                                out=outT[ds(o * P, P), ts(slot, CH)], in_=ft[:]
                            )

                # software-pipelined (slot, blk) sequence: scores of item
                # k+1 are enqueued before den/ctx of item k
                items = [
                    (slot, blk)
                    for slot in range(NSLOT)
                    for blk in range(NK[slot] // 4)
                ]
                pending = None  # (slot, blk, tiles)
                for slot, blk in items:
                    if blk == 0:
                        slot_state[slot] = {
                            "ctx": ctx_pool.tile([P, 8, CH], f32, tag="ctx",
                                                 name=f"ctx{slot}"),
                            "den": den_pool.tile([1, CH], f32, tag="den",
                                                 name=f"den{slot}"),
                        }
                    tiles = emit_scores(slot, blk)
                    if pending is not None:
                        emit_consume(pending[0], pending[1], pending[2])
                    pending = (slot, blk, tiles)
                emit_consume(pending[0], pending[1], pending[2])

    nc.compile()
    return nc


def _get_program():
    global _PROGRAM
    if _PROGRAM is None:
        _PROGRAM = _build_program()
    return _PROGRAM


def _make_in_maps(x, W_query, W_key, W_value):
    import ml_dtypes

    f8np = ml_dtypes.float8_e4m3
    pj_np = f8np if PROJ_FP8 else np.float16

    xT = np.ascontiguousarray(
        np.asarray(x, dtype=np.float32).transpose(0, 2, 1)
    )  # [B, D, S] f32

    def tile_w(w, scale, dt):
        # [o, d] -> [p, d_slab, o]
        wt = (np.asarray(w, dtype=np.float32).T * scale).astype(dt)
        return np.ascontiguousarray(wt.reshape(8, P, D).transpose(1, 0, 2))

    def tile_x(xt, nch, dt):
        # [d, s] -> [chunk, p, d_slab, s_off]
        return np.ascontiguousarray(
            xt.astype(dt).reshape(8, P, nch, CH).transpose(2, 1, 0, 3)
        )

    wqT = tile_w(W_query, WSCALE, pj_np)
    # wkT: [P, d_slab, o] -> [o_blk, P, d_slab, 128]
    wkT_flat = tile_w(W_key, WSCALE, pj_np)  # [P, 8, D]
    wkT = np.ascontiguousarray(
        wkT_flat.reshape(P, 8, 8, P).transpose(2, 0, 1, 3)
    )
    wvT = tile_w(W_value, 1.0, np.float16)
    cmat_h = []
    for h in range(2):
        cmx = np.zeros((P, CW), np.float16)
        cmx[:, C_DMAT:C_DMAT + CH] = (
            np.arange(P, dtype=np.float32)[:, None]
            - np.arange(CH, dtype=np.float32)[None, :]
        ).astype(np.float16)
        for slot in range(NSLOT):
            cid = CHUNKS_H[h][slot]
            for j in range(NK[slot]):
                cmx[:, C_AMAT + SLOTBASE[slot] + j] = CH * cid - P * j
        cmx[:, C_ONEK] = 1.0
        cmx[0, C_ONER:C_ONER + P] = 1.0
        cmat_h.append(cmx)

    NKC = 4 if USE_GATHER else 8
    in_maps = []
    for core in range(8):
        b, h = core // 2, core % 2
        xq_cols = np.concatenate(
            [np.arange(c * CH, (c + 1) * CH) for c in CHUNKS_H[h]]
        )
        xqT_b = tile_x(np.ascontiguousarray(xT[b][:, xq_cols]), NSLOT, pj_np)
        if USE_GATHER:
            kv_cols = np.arange(4 * h * CH, 4 * (h + 1) * CH)
            xkv = np.ascontiguousarray(xT[b][:, kv_cols])
        else:
            xkv = xT[b]
        xk_t = tile_x(xkv, NKC, pj_np)  # [chunk, P, 8, CH]
        # -> [chunk, dp_pair, P, 2, CH]
        xk_t = np.ascontiguousarray(
            xk_t.reshape(NKC, P, 4, 2, CH).transpose(0, 2, 1, 3, 4)
        )
        in_maps.append(
            {
                "xkT": xk_t,
                "xvT": tile_x(xkv, NKC, np.float16),
                "xqT": xqT_b,
                "wqT": wqT,
                "wkT": wkT,
                "wvT": wvT,
                "cmat": cmat_h[h],
            }
        )
    return in_maps


def _assemble(results):
    out = np.empty((B, S, D), np.float32)
    for core in range(8):
        b, h = core // 2, core % 2
        oT = np.asarray(results[core]["outT"])  # [D, NQ]
        for slot, c in enumerate(CHUNKS_H[h]):
            out[b, c * CH : (c + 1) * CH, :] = oT[:, slot * CH : (slot + 1) * CH].T
    return out


def run(inputs, trace=False, trace_cores=None):
    """Run the kernel; returns (output, BassKernelResults)."""
    from concourse.bass_utils import run_bass_kernel_spmd

    nc = _get_program()
    in_maps = _make_in_maps(
        inputs["x"], inputs["W_query"], inputs["W_key"], inputs["W_value"]
    )
    kw = {}
    if trace:
        kw = dict(trace=True, trace_cores=trace_cores, stitch_traces=False)
    res = run_bass_kernel_spmd(nc, in_maps, list(range(8)), **kw)
    return _assemble(res.results), res


def kernel(x, W_query, W_key, W_value):
    out, _ = run({"x": x, "W_query": W_query, "W_key": W_key, "W_value": W_value})
    return out


# revision 34
# speedup vs baseline: 1.0245x; 1.0245x over previous
"""Causal single-head attention (B=4, S=4096, D=1024) on 8 TRN2 NeuronCores.

Sharding: core = (batch b, half h).  Each core computes attention output for
2048 queries of one batch: query chunks {0,3,4,7} (h=0) or {1,2,5,6} (h=1) of
8x512, which balances causal work.

Design (measured 425,034ns max-core / 405,727ns mean on a healthy fleet,
vs 654,126ns fp16 baseline; absmax-rel err 1.81e-2 < 2e-2 gate):
  * Q/K projections and the QK^T scores matmul run in fp8e4 with
    MatmulPerfMode.DoubleRow (2 contraction sub-tiles per instruction;
    measured issue rate ~216-260ns per 256-deep x 512-wide MM, ~2x fp16).
    W_q/W_k are pre-scaled by 32 on the host; the extra 32*32 is folded
    into the exp scale (2^-15).  V projection / P*V context matmul stay
    fp16: fp8 V fails the 2e-2 gate (absmax-rel ~5e-2 in numpy sim).
  * K^T and V projections are deduplicated across the core pair sharing a
    batch: core (b,h) projects key chunks 4h..4h+3 only; AllGather
    collectives over pairs {2b,2b+1} exchange K^T (fp8, two half-gathers
    issued as their inputs land, each followed by contiguous readbacks)
    and V (fp16, two 2MB gathers) through DRAM, overlapped with the
    remaining projection work so attention never waits on peer data.
  * K^T lives in 8 per-chunk SBUF tiles [P,8,CH] so the post-collective
    readback DMA is contiguous per partition.
  * Startup: wk is loaded as 8 per-o-block pieces and xk chunks as 4
    dp-pair pieces so the first projection matmul starts as soon as the
    first ~256KB lands; all small constants ride in ONE merged [P,768]
    DMA; a memset-fed stream of dummy matmuls warms the PE HAM clock gate
    (cold 1.2GHz -> warm 2.4GHz).
  * The attention block loop is software-pipelined (scores of block k+1
    are enqueued on the Tensor queue before den/ctx of block k) so the
    exp->mask latency never stalls the PE; each slot's final block fuses
    the epilogue, taking 1/den on the [1,512] row BEFORE broadcasting it
    through the ones-matmul, and the output mult reads the broadcast
    directly from PSUM — keeping the kernel tail short.
Scores are computed in the S^T = [k, q] layout so no on-device transposes
are needed anywhere; P = exp(s*scale) * causal_mask with the mask built
from dmat/amat per-core data (mask = (iota_k - iota_q) <= a[slot,j]).
"""

import sys

for _p in ("/opt/trn_rl_repo",):
    if _p not in sys.path:
        sys.path.insert(0, _p)

import numpy as np

B, S, D = 4, 4096, 1024
P = 128
CH = 512                       # query chunk
NSLOT = 4                      # chunks per core
NQ = NSLOT * CH                # queries per core
NK = [8, 16, 24, 32]           # k-tiles per slot (uniform across cores)
SLOTBASE = [0, 8, 24, 48]      # amat column base per slot
CHUNKS_H = [[0, 3, 4, 7], [1, 2, 5, 6]]
WSCALE = 32.0                  # host pre-scale on W_q/W_k
SCALE = 1.0 / 32.0 / (WSCALE * WSCALE)   # exp scale = 2^-15

# merged const layout: [P, CW] fp16
C_DMAT = 0      # cols 0..511: dmat
C_AMAT = 512    # cols 512..591: amat
C_ONEK = 592    # col 592: ones (ones_k)
C_ONER = 608    # row 0, cols 608..735: ones (ones_r)
CW = 768

PROJ_FP8 = True                # Q/K projections fp8 DoubleRow
SCORES_FP8 = True              # QK^T scores fp8 DoubleRow
USE_GATHER = True              # dedup K/V projection across core pairs
WARMUP_MM = 16

_PROGRAM = None


def _build_program():
    import concourse.bass as bass
    import concourse.tile as tile
    import concourse.mybir as mybir
    from concourse import bacc
    from concourse.bass import ds, ts

    f32 = mybir.dt.float32
    f16 = mybir.dt.float16
    f8 = mybir.dt.float8e4
    DR = mybir.MatmulPerfMode.DoubleRow

    pj_dt = f8 if PROJ_FP8 else f16
    sc_dt = f8 if SCORES_FP8 else f16

    nc = bacc.Bacc(trn_type="TRN2", target_bir_lowering=False, debug=False,
                   num_devices=8)

    NKC = 4 if USE_GATHER else 8   # key chunks projected locally
    # wkT: [o_blk, P, d_slab, P]; xkT: [chunk, dp_pair, P, 2, CH]
    wkT = nc.declare_dram_parameter("wkT", [8, P, 8, P], pj_dt, isOutput=False)
    xkT = nc.declare_dram_parameter("xkT", [NKC, 4, P, 2, CH], pj_dt,
                                    isOutput=False)
    xvT = nc.declare_dram_parameter("xvT", [NKC, P, 8, CH], f16, isOutput=False)
    xqT = nc.declare_dram_parameter("xqT", [NSLOT, P, 8, CH], pj_dt,
                                    isOutput=False)
    wqT = nc.declare_dram_parameter("wqT", [P, 8, D], pj_dt, isOutput=False)
    wvT = nc.declare_dram_parameter("wvT", [P, 8, D], f16, isOutput=False)
    cmat = nc.declare_dram_parameter("cmat", [P, CW], f16, isOutput=False)
    outT = nc.declare_dram_parameter("outT", [D, NQ], f32, isOutput=True)

    if USE_GATHER:
        kx_in = nc.dram_tensor("kx_in", [4, P, 8, CH], sc_dt)
        kx_outA = nc.dram_tensor("kx_outA", [2, 2, P, 8, CH], sc_dt)
        kx_outB = nc.dram_tensor("kx_outB", [2, 2, P, 8, CH], sc_dt)
        v_in = nc.dram_tensor("v_in", [4, CH, D], f16)
        v_outA = nc.dram_tensor("v_outA", [2, 2, CH, D], f16)
        v_outB = nc.dram_tensor("v_outB", [2, 2, CH, D], f16)
        RGS = [[0, 1], [2, 3], [4, 5], [6, 7]]
        def RGrot(k):
            return RGS[k % 4:] + RGS[:k % 4]
    else:
        vscr = nc.dram_tensor("v_scratch", [S, D], f16)

    Exp = mybir.ActivationFunctionType.Exp
    is_le = mybir.AluOpType.is_le
    mult = mybir.AluOpType.mult
    bypass = mybir.AluOpType.bypass

    def v_src(j):
        """AP of V rows [j*128, j*128+128) in the gathered layout."""
        cid = j // 4
        if not USE_GATHER:
            return vscr[ds(j * P, P), :]
        r, i = cid // 4, cid % 4
        vo = v_outA if i < 2 else v_outB
        return vo[r][i % 2, ds((j % 4) * P, P), :]

    with tile.TileContext(nc, pool_alloc_mode="queue") as tc:
        with (
            tc.tile_pool(name="kt", bufs=1) as kt_pool,
            tc.tile_pool(name="qt", bufs=1) as qt_pool,
            tc.tile_pool(name="const", bufs=1) as const_pool,
        ):
            # K^T piece tiles: KTpc[cid] = key chunk cid
            KTpc = [
                kt_pool.tile([P, 8, CH], sc_dt, tag=f"kt{c}", name=f"KTpc{c}")
                for c in range(8)
            ]
            QTs = [
                qt_pool.tile([P, 8, CH], sc_dt, tag=f"qt{i}", name=f"QTs{i}")
                for i in range(NSLOT)
            ]
            cm = const_pool.tile([P, CW], f16, tag="cmat")
            wtile = const_pool.tile([P, CH], f16, tag="warm")
            dmat_sb = cm[:, ds(C_DMAT, CH)]
            amat_sb = cm[:, ds(C_AMAT, 80)]
            ones_k_sb = cm[:, ds(C_ONEK, 1)]
            ones_r_sb = cm[ds(0, 1), ds(C_ONER, P)]

            # warmup: memset-fed dummy matmuls warm the PE HAM clock gate
            nc.vector.memset(wtile[:], 1.0)

            # ---------- Phase 0+1: projections K, V, Q ---------------------
            with (
                tc.tile_pool(name="w0", bufs=1) as w_pool,
                tc.tile_pool(name="xk", bufs=4) as xk_pool,
                tc.tile_pool(name="xv", bufs=4) as xv_pool,
                tc.tile_pool(name="xq", bufs=4) as xq_pool,
                tc.tile_pool(name="kb", bufs=2) as kb_pool,
                tc.tile_pool(name="vb", bufs=6) as vb_pool,
                tc.tile_pool(name="ps0", bufs=4, space="PSUM") as ps_pool,
                tc.tile_pool(name="wps", bufs=1, space="PSUM") as wu_pool,
            ):
                # xk chunks split across sync (0,1) and scalar (2,3) so
                # kproj is never starved by a single DMA queue; dp-pair
                # pieces so the first matmul starts on the first ~128KB
                xk_tiles = []
                for c in range(4):
                    xk = xk_pool.tile([P, 8, CH], pj_dt, tag="xk",
                                      name=f"xk{c}")
                    eng = nc.sync if c < 2 else nc.scalar
                    for dp in range(4):
                        eng.dma_start(out=xk[:, ds(2 * dp, 2), :],
                                      in_=xkT[c, dp])
                    xk_tiles.append(xk)
                nc.sync.dma_start(out=cm[:], in_=cmat[:])
                # wk: 8 per-o-block pieces on gpsimd, o ascending
                wk_os = []
                for o in range(8):
                    wko = w_pool.tile([P, 8, P], pj_dt, tag=f"wk{o}",
                                      name=f"wk{o}")
                    nc.gpsimd.dma_start(out=wko[:], in_=wkT[o])
                    wk_os.append(wko)
                wv = w_pool.tile([P, 8, D], f16, tag="wv")
                wq = w_pool.tile([P, 8, D], pj_dt, tag="wq")
                nc.scalar.dma_start(out=wv[:], in_=wvT[:])

                for _ in range(WARMUP_MM):
                    wps = wu_pool.tile([1, CH], f32, name="warm")
                    nc.tensor.matmul(
                        wps[:], lhsT=wtile[:, ds(0, 1)], rhs=wtile[:],
                        start=True, stop=True,
                    )

                def kproj_mms(ps, o, xk):
                    if PROJ_FP8:
                        for dp in range(4):
                            nc.tensor.matmul(
                                ps[:],
                                lhsT=wk_os[o][:, ds(2 * dp, 2), :],
                                rhs=xk[:, ds(2 * dp, 2), :],
                                start=(dp == 0),
                                stop=(dp == 3),
                                perf_mode=DR,
                            )
                    else:
                        for d in range(8):
                            nc.tensor.matmul(
                                ps[:],
                                lhsT=wk_os[o][:, d, :],
                                rhs=xk[:, d, :],
                                start=(d == 0),
                                stop=(d == 7),
                            )

                with nc.named_scope("kproj"):
                    for c in range(NKC):
                        if c < 4:
                            xk = xk_tiles[c]
                        else:
                            xk = xk_pool.tile(
                                [P, 8, CH], pj_dt, tag="xk", name=f"xk{c}"
                            )
                            for dp in range(4):
                                nc.sync.dma_start(
                                    out=xk[:, ds(2 * dp, 2), :], in_=xkT[c, dp]
                                )
                        if USE_GATHER:
                            kb = kb_pool.tile(
                                [P, 8, CH], sc_dt, tag="kb", name=f"kb{c}"
                            )
                        for o in range(8):
                            ps = ps_pool.tile([P, CH], f32, tag="ps", name="psk")
                            kproj_mms(ps, o, xk)
                            if USE_GATHER:
                                nc.vector.tensor_copy(kb[:, o, :], ps[:])
                            else:
                                nc.vector.tensor_copy(KTpc[c][:, o, :], ps[:])
                        if USE_GATHER:
                            nc.sync.dma_start(out=kx_in[c], in_=kb[:])
                            if c == 1 or c == 3:
                                # two half-gathers: the first (chunks 0,1 +
                                # peer 4,5) completes ~30us earlier than a
                                # merged gather, unblocking attention start
                                kxo = kx_outA if c == 1 else kx_outB
                                lo = 0 if c == 1 else 2
                                nc.gpsimd.collective_compute(
                                    "AllGather", bypass,
                                    RGrot(0 if c == 1 else 1),
                                    ins=[kx_in[ds(lo, 2)].opt()],
                                    outs=[kxo[ds(0, 2)].opt()],
                                )
                                for r in range(2):
                                    for i in range(2):
                                        nc.gpsimd.dma_start(
                                            out=KTpc[4 * r + lo + i][:],
                                            in_=kxo[r][i],
                                        )

                # wq/xq ride the sync queue behind the compute-gated kb
                # writes: they are needed only at qproj (~105us) and this
                # keeps them out of the startup bandwidth window
                nc.sync.dma_start(out=wq[:], in_=wqT[:])
                xq_tiles = []
                for s in range(NSLOT):
                    xq = xq_pool.tile([P, 8, CH], pj_dt, tag="xq", name=f"xq{s}")
                    nc.sync.dma_start(out=xq[:], in_=xqT[s])
                    xq_tiles.append(xq)

                with nc.named_scope("vproj"):
                    for c in range(NKC):
                        xv = xv_pool.tile(
                            [P, 8, CH], f16, tag="xv", name=f"xv{c}"
                        )
                        nc.sync.dma_start(out=xv[:], in_=xvT[c])
                        for kt_i in range(4):
                            vb = vb_pool.tile([P, D], f16, tag="vb", name="vb")
                            for oh in range(2):
                                ps = ps_pool.tile(
                                    [P, CH], f32, tag="ps", name="psv"
                                )
                                for d in range(8):
                                    nc.tensor.matmul(
                                        ps[:],
                                        lhsT=xv[:, d, ts(kt_i, P)],
                                        rhs=wv[:, d, ts(oh, CH)],
                                        start=(d == 0),
                                        stop=(d == 7),
                                    )
                                # DVE is idle during vproj; the ACT-engine
                                # copy measured ~1.7us each and serialized
                                # the whole phase through the psum pool
                                nc.vector.tensor_copy(vb[:, ts(oh, CH)], ps[:])
                            if USE_GATHER:
                                # scalar queue is idle here; sync is busy
                                # with kb writes — fast drain keeps the
                                # 3-deep vb pool recycling
                                nc.scalar.dma_start(
                                    out=v_in[c][ds(kt_i * P, P), :], in_=vb[:]
                                )
                            else:
                                nc.scalar.dma_start(
                                    out=vscr[ds(c * CH + kt_i * P, P), :],
                                    in_=vb[:],
                                )
                        if USE_GATHER and c == 1:
                            nc.gpsimd.collective_compute(
                                "AllGather", bypass, RGrot(2),
                                ins=[v_in[ds(0, 2)].opt()],
                                outs=[v_outA[ds(0, 2)].opt()],
                            )
                    if USE_GATHER:
                        nc.gpsimd.collective_compute(
                            "AllGather", bypass, RGrot(3),
                            ins=[v_in[ds(2, 2)].opt()],
                            outs=[v_outB[ds(0, 2)].opt()],
                        )

                with nc.named_scope("qproj"):
                    for s in range(NSLOT):
                        for o in range(8):
                            ps = ps_pool.tile([P, CH], f32, tag="ps", name="psq")
                            if PROJ_FP8:
                                for dp in range(4):
                                    nc.tensor.matmul(
                                        ps[:],
                                        lhsT=wq[:, ds(2 * dp, 2), ts(o, P)],
                                        rhs=xq_tiles[s][:, ds(2 * dp, 2), :],
                                        start=(dp == 0),
                                        stop=(dp == 3),
                                        perf_mode=DR,
                                    )
                            else:
                                for d in range(8):
                                    nc.tensor.matmul(
                                        ps[:],
                                        lhsT=wq[:, d, ts(o, P)],
                                        rhs=xq_tiles[s][:, d, :],
                                        start=(d == 0),
                                        stop=(d == 7),
                                    )
                            nc.vector.tensor_copy(QTs[s][:, o, :], ps[:])

            # ---------------- Phase 2: attention ---------------------------
            with (
                tc.tile_pool(name="ctx", bufs=2) as ctx_pool,
                tc.tile_pool(name="vt", bufs=12) as v_pool,
                tc.tile_pool(name="pt", bufs=12) as p_pool,
                tc.tile_pool(name="et", bufs=3) as e_pool,
                tc.tile_pool(name="fo", bufs=4) as f_pool,
                tc.tile_pool(name="dsb", bufs=4) as den_pool,
                tc.tile_pool(name="pss", bufs=3, space="PSUM") as s_ps_pool,
                tc.tile_pool(name="psc", bufs=3, space="PSUM") as c_ps_pool,
                tc.tile_pool(name="psd", bufs=1, space="PSUM") as d_ps_pool,
                tc.tile_pool(name="psb", bufs=1, space="PSUM") as b_ps_pool,
                nc.named_scope("attn"),
            ):
                slot_state = {}

                def emit_scores(slot, blk):
                    """Load V tiles, compute scores -> exp -> mask for one
                    4-k-tile block.  Returns (p_tiles, v_tiles)."""
                    p_tiles, v_tiles = [], []
                    for j4 in range(4):
                        j = blk * 4 + j4
                        vt = v_pool.tile([P, D], f16, tag="vt", name="vt")
                        nc.scalar.dma_start(out=vt[:], in_=v_src(j))
                        KT = KTpc[j // 4]
                        sps = s_ps_pool.tile([P, CH], f32, name="sps")
                        if SCORES_FP8:
                            for op in range(4):
                                nc.tensor.matmul(
                                    sps[:],
                                    lhsT=KT[:, ds(2 * op, 2), ds((j % 4) * P, P)],
                                    rhs=QTs[slot][:, ds(2 * op, 2), :],
                                    start=(op == 0),
                                    stop=(op == 3),
                                    perf_mode=DR,
                                )
                        else:
                            for o in range(8):
                                nc.tensor.matmul(
                                    sps[:],
                                    lhsT=KT[:, o, ds((j % 4) * P, P)],
                                    rhs=QTs[slot][:, o, :],
                                    start=(o == 0),
                                    stop=(o == 7),
                                )
                        et = e_pool.tile([P, CH], f16, tag="et", name="et")
                        nc.scalar.activation(et[:], sps[:], Exp, scale=SCALE)
                        pt = p_pool.tile([P, CH], f16, tag="pt", name="pt")
                        col = SLOTBASE[slot] + j
                        nc.vector.scalar_tensor_tensor(
                            out=pt[:],
                            in0=dmat_sb,
                            scalar=amat_sb[:, ds(col, 1)],
                            in1=et[:],
                            op0=is_le,
                            op1=mult,
                        )
                        p_tiles.append(pt)
                        v_tiles.append(vt)
                    return p_tiles, v_tiles

                def emit_consume(slot, blk, tiles):
                    """den + ctx matmuls for a block; on the slot's final
                    block, fuse the epilogue (reciprocal, out mult + DMA)."""
                    p_tiles, v_tiles = tiles
                    final = blk == NK[slot] // 4 - 1
                    st = slot_state[slot]
                    ctx, den = st["ctx"], st["den"]
                    dps = d_ps_pool.tile([1, CH], f32, name="dps")
                    for j4 in range(4):
                        nc.tensor.matmul(
                            dps[:],
                            lhsT=ones_k_sb,
                            rhs=p_tiles[j4][:],
                            start=(j4 == 0),
                            stop=(j4 == 3),
                        )
                    if blk == 0:
                        nc.vector.tensor_copy(den[:], dps[:])
                    else:
                        nc.vector.tensor_add(den[:], den[:], dps[:])
                    if final:
                        # reciprocal on the [1,CH] row (cheap), then
                        # broadcast the reciprocal across partitions with
                        # the ones_r matmul; mults read the PSUM directly
                        rrow = den_pool.tile([1, CH], f32, tag="rrow",
                                             name="rrow")
                        nc.vector.reciprocal(rrow[:], den[:])
                        r16 = den_pool.tile([1, CH], f16, tag="r16",
                                            name="r16")
                        nc.vector.tensor_copy(r16[:], rrow[:])
                        bps = b_ps_pool.tile([P, CH], f32, name="bps")
                        nc.tensor.matmul(
                            bps[:], lhsT=ones_r_sb, rhs=r16[:],
                            start=True, stop=True,
                        )
                    for o in range(8):
                        cps = c_ps_pool.tile([P, CH], f32, name="cps")
                        for j4 in range(4):
                            nc.tensor.matmul(
                                cps[:],
                                lhsT=v_tiles[j4][:, ts(o, P)],
                                rhs=p_tiles[j4][:],
                                start=(j4 == 0),
                                stop=(j4 == 3),
                            )
                        if blk == 0:
                            nc.vector.tensor_copy(ctx[:, o, :], cps[:])
                        else:
                            nc.vector.tensor_add(
                                ctx[:, o, :], ctx[:, o, :], cps[:]
                            )
                        if final:
                            ft = f_pool.tile([P, CH], f32, tag="ft", name="ft")
                            nc.vector.tensor_mul(ft[:], ctx[:, o, :], bps[:])
                            nc.sync.dma_start(
                                out=outT[ds(o * P, P), ts(slot, CH)], in_=ft[:]
                            )

                # software-pipelined (slot, blk) sequence: scores of item
                # k+1 are enqueued before den/ctx of item k
                items = [
                    (slot, blk)
                    for slot in range(NSLOT)
                    for blk in range(NK[slot] // 4)
                ]
                pending = None  # (slot, blk, tiles)
                for slot, blk in items:
                    if blk == 0:
                        slot_state[slot] = {
                            "ctx": ctx_pool.tile([P, 8, CH], f32, tag="ctx",
                                                 name=f"ctx{slot}"),
                            "den": den_pool.tile([1, CH], f32, tag="den",
                                                 name=f"den{slot}"),
                        }
                    tiles = emit_scores(slot, blk)
                    if pending is not None:
                        emit_consume(pending[0], pending[1], pending[2])
                    pending = (slot, blk, tiles)
                emit_consume(pending[0], pending[1], pending[2])

    nc.compile()
    return nc


def _get_program():
    global _PROGRAM
    if _PROGRAM is None:
        _PROGRAM = _build_program()
    return _PROGRAM


def _make_in_maps(x, W_query, W_key, W_value):
    import ml_dtypes

    f8np = ml_dtypes.float8_e4m3
    pj_np = f8np if PROJ_FP8 else np.float16

    xT = np.ascontiguousarray(
        np.asarray(x, dtype=np.float32).transpose(0, 2, 1)
    )  # [B, D, S] f32

    def tile_w(w, scale, dt):
        # [o, d] -> [p, d_slab, o]
        wt = (np.asarray(w, dtype=np.float32).T * scale).astype(dt)
        return np.ascontiguousarray(wt.reshape(8, P, D).transpose(1, 0, 2))

    def tile_x(xt, nch, dt):
        # [d, s] -> [chunk, p, d_slab, s_off]
        return np.ascontiguousarray(
            xt.astype(dt).reshape(8, P, nch, CH).transpose(2, 1, 0, 3)
        )

    wqT = tile_w(W_query, WSCALE, pj_np)
    # wkT: [P, d_slab, o] -> [o_blk, P, d_slab, 128]
    wkT_flat = tile_w(W_key, WSCALE, pj_np)  # [P, 8, D]
    wkT = np.ascontiguousarray(
        wkT_flat.reshape(P, 8, 8, P).transpose(2, 0, 1, 3)
    )
    wvT = tile_w(W_value, 1.0, np.float16)
    cmat_h = []
    for h in range(2):
        cmx = np.zeros((P, CW), np.float16)
        cmx[:, C_DMAT:C_DMAT + CH] = (
            np.arange(P, dtype=np.float32)[:, None]
            - np.arange(CH, dtype=np.float32)[None, :]
        ).astype(np.float16)
        for slot in range(NSLOT):
            cid = CHUNKS_H[h][slot]
            for j in range(NK[slot]):
                cmx[:, C_AMAT + SLOTBASE[slot] + j] = CH * cid - P * j
        cmx[:, C_ONEK] = 1.0
        cmx[0, C_ONER:C_ONER + P] = 1.0
        cmat_h.append(cmx)

    NKC = 4 if USE_GATHER else 8
    in_maps = []
    for core in range(8):
        b, h = core // 2, core % 2
        xq_cols = np.concatenate(
            [np.arange(c * CH, (c + 1) * CH) for c in CHUNKS_H[h]]
        )
        xqT_b = tile_x(np.ascontiguousarray(xT[b][:, xq_cols]), NSLOT, pj_np)
        if USE_GATHER:
            kv_cols = np.arange(4 * h * CH, 4 * (h + 1) * CH)
            xkv = np.ascontiguousarray(xT[b][:, kv_cols])
        else:
            xkv = xT[b]
        xk_t = tile_x(xkv, NKC, pj_np)  # [chunk, P, 8, CH]
        # -> [chunk, dp_pair, P, 2, CH]
        xk_t = np.ascontiguousarray(
            xk_t.reshape(NKC, P, 4, 2, CH).transpose(0, 2, 1, 3, 4)
        )
        in_maps.append(
            {
                "xkT": xk_t,
                "xvT": tile_x(xkv, NKC, np.float16),
                "xqT": xqT_b,
                "wqT": wqT,
                "wkT": wkT,
                "wvT": wvT,
                "cmat": cmat_h[h],
            }
        )
    return in_maps


def _assemble(results):
    out = np.empty((B, S, D), np.float32)
    for core in range(8):
        b, h = core // 2, core % 2
        oT = np.asarray(results[core]["outT"])  # [D, NQ]
        for slot, c in enumerate(CHUNKS_H[h]):
            out[b, c * CH : (c + 1) * CH, :] = oT[:, slot * CH : (slot + 1) * CH].T
    return out


def run(inputs, trace=False, trace_cores=None):
    """Run the kernel; returns (output, BassKernelResults)."""
    from concourse.bass_utils import run_bass_kernel_spmd

    nc = _get_program()
    in_maps = _make_in_maps(
        inputs["x"], inputs["W_query"], inputs["W_key"], inputs["W_value"]
    )
    kw = {}
    if trace:
        kw = dict(trace=True, trace_cores=trace_cores, stitch_traces=False)
    res = run_bass_kernel_spmd(nc, in_maps, list(range(8)), **kw)
    return _assemble(res.results), res


def kernel(x, W_query, W_key, W_value):
    out, _ = run({"x": x, "W_query": W_query, "W_key": W_key, "W_value": W_value})
    return out


# revision 35
# speedup vs baseline: 1.0250x; 1.0005x over previous
"""Causal single-head attention (B=4, S=4096, D=1024) on 8 TRN2 NeuronCores.

Sharding: core = (batch b, half h).  Each core computes attention output for
2048 queries of one batch: query chunks {0,3,4,7} (h=0) or {1,2,5,6} (h=1) of
8x512, which balances causal work.

Design (measured 414,866ns max-core / 406,250ns mean on a healthy fleet,
vs 654,126ns fp16 baseline; absmax-rel err 1.81e-2 < 2e-2 gate):
  * Q/K projections and the QK^T scores matmul run in fp8e4 with
    MatmulPerfMode.DoubleRow (2 contraction sub-tiles per instruction;
    measured issue rate ~216-260ns per 256-deep x 512-wide MM, ~2x fp16).
    W_q/W_k are pre-scaled by 32 on the host; the extra 32*32 is folded
    into the exp scale (2^-15).  V projection / P*V context matmul stay
    fp16: fp8 V fails the 2e-2 gate (absmax-rel ~5e-2 in numpy sim).
  * K^T and V projections are deduplicated across the core pair sharing a
    batch: core (b,h) projects key chunks 4h..4h+3 only; AllGather
    collectives over pairs {2b,2b+1} exchange K^T (fp8, two half-gathers
    issued as their inputs land, each followed by contiguous readbacks)
    and V (fp16, two 2MB gathers) through DRAM, overlapped with the
    remaining projection work.  Replica-group list order is ROTATED per
    collective: group execution is order-serial, so rotation spreads the
    served-last penalty across pairs instead of one pair eating it 4x.
  * K^T lives in 8 per-chunk SBUF tiles [P,8,CH] so the post-collective
    readback DMA is contiguous per partition.
  * Startup: wk is loaded as 8 per-o-block pieces and xk chunks as 4
    dp-pair pieces so the first projection matmul starts as soon as the
    first ~256KB lands; all small constants ride in ONE merged [P,768]
    DMA; a memset-fed stream of dummy matmuls warms the PE HAM clock gate
    (cold 1.2GHz -> warm 2.4GHz).
  * The attention block loop is software-pipelined (scores of block k+1
    are enqueued on the Tensor queue before den/ctx of block k) so the
    exp->mask latency never stalls the PE; each slot's final block fuses
    the epilogue, taking 1/den on the [1,512] row BEFORE broadcasting it
    through the ones-matmul, and the output mult reads the broadcast
    directly from PSUM — keeping the kernel tail short.
Scores are computed in the S^T = [k, q] layout so no on-device transposes
are needed anywhere; P = exp(s*scale) * causal_mask with the mask built
from dmat/amat per-core data (mask = (iota_k - iota_q) <= a[slot,j]).
"""

import sys

for _p in ("/opt/trn_rl_repo",):
    if _p not in sys.path:
        sys.path.insert(0, _p)

import numpy as np

B, S, D = 4, 4096, 1024
P = 128
CH = 512                       # query chunk
NSLOT = 4                      # chunks per core
NQ = NSLOT * CH                # queries per core
NK = [8, 16, 24, 32]           # k-tiles per slot (uniform across cores)
SLOTBASE = [0, 8, 24, 48]      # amat column base per slot
CHUNKS_H = [[0, 3, 4, 7], [1, 2, 5, 6]]
WSCALE = 32.0                  # host pre-scale on W_q/W_k
SCALE = 1.0 / 32.0 / (WSCALE * WSCALE)   # exp scale = 2^-15

# merged const layout: [P, CW] fp16
C_DMAT = 0      # cols 0..511: dmat
C_AMAT = 512    # cols 512..591: amat
C_ONEK = 592    # col 592: ones (ones_k)
C_ONER = 608    # row 0, cols 608..735: ones (ones_r)
CW = 768

PROJ_FP8 = True                # Q/K projections fp8 DoubleRow
SCORES_FP8 = True              # QK^T scores fp8 DoubleRow
USE_GATHER = True              # dedup K/V projection across core pairs
WARMUP_MM = 16

_PROGRAM = None


def _build_program():
    import concourse.bass as bass
    import concourse.tile as tile
    import concourse.mybir as mybir
    from concourse import bacc
    from concourse.bass import ds, ts

    f32 = mybir.dt.float32
    f16 = mybir.dt.float16
    f8 = mybir.dt.float8e4
    DR = mybir.MatmulPerfMode.DoubleRow

    pj_dt = f8 if PROJ_FP8 else f16
    sc_dt = f8 if SCORES_FP8 else f16

    nc = bacc.Bacc(trn_type="TRN2", target_bir_lowering=False, debug=False,
                   num_devices=8)

    NKC = 4 if USE_GATHER else 8   # key chunks projected locally
    # wkT: [o_blk, P, d_slab, P]; xkT: [chunk, dp_pair, P, 2, CH]
    wkT = nc.declare_dram_parameter("wkT", [8, P, 8, P], pj_dt, isOutput=False)
    xkT = nc.declare_dram_parameter("xkT", [NKC, 4, P, 2, CH], pj_dt,
                                    isOutput=False)
    xvT = nc.declare_dram_parameter("xvT", [NKC, P, 8, CH], f16, isOutput=False)
    xqT = nc.declare_dram_parameter("xqT", [NSLOT, P, 8, CH], pj_dt,
                                    isOutput=False)
    wqT = nc.declare_dram_parameter("wqT", [P, 8, D], pj_dt, isOutput=False)
    wvT = nc.declare_dram_parameter("wvT", [P, 8, D], f16, isOutput=False)
    cmat = nc.declare_dram_parameter("cmat", [P, CW], f16, isOutput=False)
    outT = nc.declare_dram_parameter("outT", [D, NQ], f32, isOutput=True)

    if USE_GATHER:
        kx_in = nc.dram_tensor("kx_in", [4, P, 8, CH], sc_dt)
        kx_outA = nc.dram_tensor("kx_outA", [2, 2, P, 8, CH], sc_dt)
        kx_outB = nc.dram_tensor("kx_outB", [2, 2, P, 8, CH], sc_dt)
        v_in = nc.dram_tensor("v_in", [4, CH, D], f16)
        v_outA = nc.dram_tensor("v_outA", [2, 2, CH, D], f16)
        v_outB = nc.dram_tensor("v_outB", [2, 2, CH, D], f16)
        RGS = [[0, 1], [2, 3], [4, 5], [6, 7]]
        def RGrot(k):
            return RGS[k % 4:] + RGS[:k % 4]
    else:
        vscr = nc.dram_tensor("v_scratch", [S, D], f16)

    Exp = mybir.ActivationFunctionType.Exp
    is_le = mybir.AluOpType.is_le
    mult = mybir.AluOpType.mult
    bypass = mybir.AluOpType.bypass

    def v_src(j):
        """AP of V rows [j*128, j*128+128) in the gathered layout."""
        cid = j // 4
        if not USE_GATHER:
            return vscr[ds(j * P, P), :]
        r, i = cid // 4, cid % 4
        vo = v_outA if i < 2 else v_outB
        return vo[r][i % 2, ds((j % 4) * P, P), :]

    with tile.TileContext(nc, pool_alloc_mode="queue") as tc:
        with (
            tc.tile_pool(name="kt", bufs=1) as kt_pool,
            tc.tile_pool(name="qt", bufs=1) as qt_pool,
            tc.tile_pool(name="const", bufs=1) as const_pool,
        ):
            # K^T piece tiles: KTpc[cid] = key chunk cid
            KTpc = [
                kt_pool.tile([P, 8, CH], sc_dt, tag=f"kt{c}", name=f"KTpc{c}")
                for c in range(8)
            ]
            QTs = [
                qt_pool.tile([P, 8, CH], sc_dt, tag=f"qt{i}", name=f"QTs{i}")
                for i in range(NSLOT)
            ]
            cm = const_pool.tile([P, CW], f16, tag="cmat")
            wtile = const_pool.tile([P, CH], f16, tag="warm")
            dmat_sb = cm[:, ds(C_DMAT, CH)]
            amat_sb = cm[:, ds(C_AMAT, 80)]
            ones_k_sb = cm[:, ds(C_ONEK, 1)]
            ones_r_sb = cm[ds(0, 1), ds(C_ONER, P)]

            # warmup: memset-fed dummy matmuls warm the PE HAM clock gate
            nc.vector.memset(wtile[:], 1.0)

            # ---------- Phase 0+1: projections K, V, Q ---------------------
            with (
                tc.tile_pool(name="w0", bufs=1) as w_pool,
                tc.tile_pool(name="xk", bufs=4) as xk_pool,
                tc.tile_pool(name="xv", bufs=4) as xv_pool,
                tc.tile_pool(name="xq", bufs=4) as xq_pool,
                tc.tile_pool(name="kb", bufs=2) as kb_pool,
                tc.tile_pool(name="vb", bufs=6) as vb_pool,
                tc.tile_pool(name="ps0", bufs=4, space="PSUM") as ps_pool,
                tc.tile_pool(name="wps", bufs=1, space="PSUM") as wu_pool,
            ):
                # xk chunks split across sync (0,1) and scalar (2,3) so
                # kproj is never starved by a single DMA queue; dp-pair
                # pieces so the first matmul starts on the first ~128KB
                xk_tiles = []
                for c in range(4):
                    xk = xk_pool.tile([P, 8, CH], pj_dt, tag="xk",
                                      name=f"xk{c}")
                    eng = nc.sync if c < 2 else nc.scalar
                    for dp in range(4):
                        eng.dma_start(out=xk[:, ds(2 * dp, 2), :],
                                      in_=xkT[c, dp])
                    xk_tiles.append(xk)
                nc.sync.dma_start(out=cm[:], in_=cmat[:])
                # wk: 8 per-o-block pieces on gpsimd, o ascending
                wk_os = []
                for o in range(8):
                    wko = w_pool.tile([P, 8, P], pj_dt, tag=f"wk{o}",
                                      name=f"wk{o}")
                    nc.gpsimd.dma_start(out=wko[:], in_=wkT[o])
                    wk_os.append(wko)
                wv = w_pool.tile([P, 8, D], f16, tag="wv")
                wq = w_pool.tile([P, 8, D], pj_dt, tag="wq")
                nc.scalar.dma_start(out=wv[:], in_=wvT[:])

                for _ in range(WARMUP_MM):
                    wps = wu_pool.tile([1, CH], f32, name="warm")
                    nc.tensor.matmul(
                        wps[:], lhsT=wtile[:, ds(0, 1)], rhs=wtile[:],
                        start=True, stop=True,
                    )

                def kproj_mms(ps, o, xk):
                    if PROJ_FP8:
                        for dp in range(4):
                            nc.tensor.matmul(
                                ps[:],
                                lhsT=wk_os[o][:, ds(2 * dp, 2), :],
                                rhs=xk[:, ds(2 * dp, 2), :],
                                start=(dp == 0),
                                stop=(dp == 3),
                                perf_mode=DR,
                            )
                    else:
                        for d in range(8):
                            nc.tensor.matmul(
                                ps[:],
                                lhsT=wk_os[o][:, d, :],
                                rhs=xk[:, d, :],
                                start=(d == 0),
                                stop=(d == 7),
                            )

                with nc.named_scope("kproj"):
                    for c in range(NKC):
                        if c < 4:
                            xk = xk_tiles[c]
                        else:
                            xk = xk_pool.tile(
                                [P, 8, CH], pj_dt, tag="xk", name=f"xk{c}"
                            )
                            for dp in range(4):
                                nc.sync.dma_start(
                                    out=xk[:, ds(2 * dp, 2), :], in_=xkT[c, dp]
                                )
                        if USE_GATHER:
                            kb = kb_pool.tile(
                                [P, 8, CH], sc_dt, tag="kb", name=f"kb{c}"
                            )
                        for o in range(8):
                            ps = ps_pool.tile([P, CH], f32, tag="ps", name="psk")
                            kproj_mms(ps, o, xk)
                            if USE_GATHER:
                                nc.vector.tensor_copy(kb[:, o, :], ps[:])
                            else:
                                nc.vector.tensor_copy(KTpc[c][:, o, :], ps[:])
                        if USE_GATHER:
                            nc.sync.dma_start(out=kx_in[c], in_=kb[:])
                            if c == 1 or c == 3:
                                # two half-gathers: the first (chunks 0,1 +
                                # peer 4,5) completes ~30us earlier than a
                                # merged gather, unblocking attention start
                                kxo = kx_outA if c == 1 else kx_outB
                                lo = 0 if c == 1 else 2
                                nc.gpsimd.collective_compute(
                                    "AllGather", bypass,
                                    RGrot(0 if c == 1 else 1),
                                    ins=[kx_in[ds(lo, 2)].opt()],
                                    outs=[kxo[ds(0, 2)].opt()],
                                )
                                for r in range(2):
                                    for i in range(2):
                                        nc.gpsimd.dma_start(
                                            out=KTpc[4 * r + lo + i][:],
                                            in_=kxo[r][i],
                                        )

                # wq/xq ride the sync queue behind the compute-gated kb
                # writes: they are needed only at qproj (~105us) and this
                # keeps them out of the startup bandwidth window
                nc.sync.dma_start(out=wq[:], in_=wqT[:])
                xq_tiles = []
                for s in range(NSLOT):
                    xq = xq_pool.tile([P, 8, CH], pj_dt, tag="xq", name=f"xq{s}")
                    nc.sync.dma_start(out=xq[:], in_=xqT[s])
                    xq_tiles.append(xq)

                with nc.named_scope("vproj"):
                    for c in range(NKC):
                        xv = xv_pool.tile(
                            [P, 8, CH], f16, tag="xv", name=f"xv{c}"
                        )
                        nc.sync.dma_start(out=xv[:], in_=xvT[c])
                        for kt_i in range(4):
                            vb = vb_pool.tile([P, D], f16, tag="vb", name="vb")
                            for oh in range(2):
                                ps = ps_pool.tile(
                                    [P, CH], f32, tag="ps", name="psv"
                                )
                                for d in range(8):
                                    nc.tensor.matmul(
                                        ps[:],
                                        lhsT=xv[:, d, ts(kt_i, P)],
                                        rhs=wv[:, d, ts(oh, CH)],
                                        start=(d == 0),
                                        stop=(d == 7),
                                    )
                                # DVE is idle during vproj; the ACT-engine
                                # copy measured ~1.7us each and serialized
                                # the whole phase through the psum pool
                                nc.vector.tensor_copy(vb[:, ts(oh, CH)], ps[:])
                            if USE_GATHER:
                                # scalar queue is idle here; sync is busy
                                # with kb writes — fast drain keeps the
                                # 3-deep vb pool recycling
                                nc.scalar.dma_start(
                                    out=v_in[c][ds(kt_i * P, P), :], in_=vb[:]
                                )
                            else:
                                nc.scalar.dma_start(
                                    out=vscr[ds(c * CH + kt_i * P, P), :],
                                    in_=vb[:],
                                )
                        if USE_GATHER and c == 1:
                            nc.gpsimd.collective_compute(
                                "AllGather", bypass, RGrot(2),
                                ins=[v_in[ds(0, 2)].opt()],
                                outs=[v_outA[ds(0, 2)].opt()],
                            )
                    if USE_GATHER:
                        nc.gpsimd.collective_compute(
                            "AllGather", bypass, RGrot(3),
                            ins=[v_in[ds(2, 2)].opt()],
                            outs=[v_outB[ds(0, 2)].opt()],
                        )

                with nc.named_scope("qproj"):
                    for s in range(NSLOT):
                        for o in range(8):
                            ps = ps_pool.tile([P, CH], f32, tag="ps", name="psq")
                            if PROJ_FP8:
                                for dp in range(4):
                                    nc.tensor.matmul(
                                        ps[:],
                                        lhsT=wq[:, ds(2 * dp, 2), ts(o, P)],
                                        rhs=xq_tiles[s][:, ds(2 * dp, 2), :],
                                        start=(dp == 0),
                                        stop=(dp == 3),
                                        perf_mode=DR,
                                    )
                            else:
                                for d in range(8):
                                    nc.tensor.matmul(
                                        ps[:],
                                        lhsT=wq[:, d, ts(o, P)],
                                        rhs=xq_tiles[s][:, d, :],
                                        start=(d == 0),
                                        stop=(d == 7),
                                    )
                            nc.vector.tensor_copy(QTs[s][:, o, :], ps[:])

            # ---------------- Phase 2: attention ---------------------------
            with (
                tc.tile_pool(name="ctx", bufs=2) as ctx_pool,
                tc.tile_pool(name="vt", bufs=12) as v_pool,
                tc.tile_pool(name="pt", bufs=12) as p_pool,
                tc.tile_pool(name="et", bufs=3) as e_pool,
                tc.tile_pool(name="fo", bufs=4) as f_pool,
                tc.tile_pool(name="dsb", bufs=4) as den_pool,
                tc.tile_pool(name="pss", bufs=3, space="PSUM") as s_ps_pool,
                tc.tile_pool(name="psc", bufs=3, space="PSUM") as c_ps_pool,
                tc.tile_pool(name="psd", bufs=1, space="PSUM") as d_ps_pool,
                tc.tile_pool(name="psb", bufs=1, space="PSUM") as b_ps_pool,
                nc.named_scope("attn"),
            ):
                slot_state = {}

                def emit_scores(slot, blk):
                    """Load V tiles, compute scores -> exp -> mask for one
                    4-k-tile block.  Returns (p_tiles, v_tiles)."""
                    p_tiles, v_tiles = [], []
                    for j4 in range(4):
                        j = blk * 4 + j4
                        vt = v_pool.tile([P, D], f16, tag="vt", name="vt")
                        nc.scalar.dma_start(out=vt[:], in_=v_src(j))
                        KT = KTpc[j // 4]
                        sps = s_ps_pool.tile([P, CH], f32, name="sps")
                        if SCORES_FP8:
                            for op in range(4):
                                nc.tensor.matmul(
                                    sps[:],
                                    lhsT=KT[:, ds(2 * op, 2), ds((j % 4) * P, P)],
                                    rhs=QTs[slot][:, ds(2 * op, 2), :],
                                    start=(op == 0),
                                    stop=(op == 3),
                                    perf_mode=DR,
                                )
                        else:
                            for o in range(8):
                                nc.tensor.matmul(
                                    sps[:],
                                    lhsT=KT[:, o, ds((j % 4) * P, P)],
                                    rhs=QTs[slot][:, o, :],
                                    start=(o == 0),
                                    stop=(o == 7),
                                )
                        et = e_pool.tile([P, CH], f16, tag="et", name="et")
                        nc.scalar.activation(et[:], sps[:], Exp, scale=SCALE)
                        pt = p_pool.tile([P, CH], f16, tag="pt", name="pt")
                        col = SLOTBASE[slot] + j
                        nc.vector.scalar_tensor_tensor(
                            out=pt[:],
                            in0=dmat_sb,
                            scalar=amat_sb[:, ds(col, 1)],
                            in1=et[:],
                            op0=is_le,
                            op1=mult,
                        )
                        p_tiles.append(pt)
                        v_tiles.append(vt)
                    return p_tiles, v_tiles

                def emit_consume(slot, blk, tiles):
                    """den + ctx matmuls for a block; on the slot's final
                    block, fuse the epilogue (reciprocal, out mult + DMA)."""
                    p_tiles, v_tiles = tiles
                    final = blk == NK[slot] // 4 - 1
                    st = slot_state[slot]
                    ctx, den = st["ctx"], st["den"]
                    dps = d_ps_pool.tile([1, CH], f32, name="dps")
                    for j4 in range(4):
                        nc.tensor.matmul(
                            dps[:],
                            lhsT=ones_k_sb,
                            rhs=p_tiles[j4][:],
                            start=(j4 == 0),
                            stop=(j4 == 3),
                        )
                    if blk == 0:
                        nc.vector.tensor_copy(den[:], dps[:])
                    else:
                        nc.vector.tensor_add(den[:], den[:], dps[:])
                    if final:
                        # reciprocal on the [1,CH] row (cheap), then
                        # broadcast the reciprocal across partitions with
                        # the ones_r matmul; mults read the PSUM directly
                        rrow = den_pool.tile([1, CH], f32, tag="rrow",
                                             name="rrow")
                        nc.vector.reciprocal(rrow[:], den[:])
                        r16 = den_pool.tile([1, CH], f16, tag="r16",
                                            name="r16")
                        nc.vector.tensor_copy(r16[:], rrow[:])
                        bps = b_ps_pool.tile([P, CH], f32, name="bps")
                        nc.tensor.matmul(
                            bps[:], lhsT=ones_r_sb, rhs=r16[:],
                            start=True, stop=True,
                        )
                    for o in range(8):
                        cps = c_ps_pool.tile([P, CH], f32, name="cps")
                        for j4 in range(4):
                            nc.tensor.matmul(
                                cps[:],
                                lhsT=v_tiles[j4][:, ts(o, P)],
                                rhs=p_tiles[j4][:],
                                start=(j4 == 0),
                                stop=(j4 == 3),
                            )
                        if blk == 0:
                            nc.vector.tensor_copy(ctx[:, o, :], cps[:])
                        else:
                            nc.vector.tensor_add(
                                ctx[:, o, :], ctx[:, o, :], cps[:]
                            )
                        if final:
                            ft = f_pool.tile([P, CH], f32, tag="ft", name="ft")
                            nc.vector.tensor_mul(ft[:], ctx[:, o, :], bps[:])
                            nc.sync.dma_start(
                                out=outT[ds(o * P, P), ts(slot, CH)], in_=ft[:]
                            )

                # software-pipelined (slot, blk) sequence: scores of item
                # k+1 are enqueued before den/ctx of item k
                items = [
                    (slot, blk)
                    for slot in range(NSLOT)
                    for blk in range(NK[slot] // 4)
                ]
                pending = None  # (slot, blk, tiles)
                for slot, blk in items:
                    if blk == 0:
                        slot_state[slot] = {
                            "ctx": ctx_pool.tile([P, 8, CH], f32, tag="ctx",
                                                 name=f"ctx{slot}"),
                            "den": den_pool.tile([1, CH], f32, tag="den",
                                                 name=f"den{slot}"),
                        }
                    tiles = emit_scores(slot, blk)
                    if pending is not None:
                        emit_consume(pending[0], pending[1], pending[2])
                    pending = (slot, blk, tiles)
                emit_consume(pending[0], pending[1], pending[2])

    nc.compile()
    return nc


def _get_program():
    global _PROGRAM
    if _PROGRAM is None:
        _PROGRAM = _build_program()
    return _PROGRAM


def _make_in_maps(x, W_query, W_key, W_value):
    import ml_dtypes

    f8np = ml_dtypes.float8_e4m3
    pj_np = f8np if PROJ_FP8 else np.float16

    xT = np.ascontiguousarray(
        np.asarray(x, dtype=np.float32).transpose(0, 2, 1)
    )  # [B, D, S] f32

    def tile_w(w, scale, dt):
        # [o, d] -> [p, d_slab, o]
        wt = (np.asarray(w, dtype=np.float32).T * scale).astype(dt)
        return np.ascontiguousarray(wt.reshape(8, P, D).transpose(1, 0, 2))

    def tile_x(xt, nch, dt):
        # [d, s] -> [chunk, p, d_slab, s_off]
        return np.ascontiguousarray(
            xt.astype(dt).reshape(8, P, nch, CH).transpose(2, 1, 0, 3)
        )

    wqT = tile_w(W_query, WSCALE, pj_np)
    # wkT: [P, d_slab, o] -> [o_blk, P, d_slab, 128]
    wkT_flat = tile_w(W_key, WSCALE, pj_np)  # [P, 8, D]
    wkT = np.ascontiguousarray(
        wkT_flat.reshape(P, 8, 8, P).transpose(2, 0, 1, 3)
    )
    wvT = tile_w(W_value, 1.0, np.float16)
    cmat_h = []
    for h in range(2):
        cmx = np.zeros((P, CW), np.float16)
        cmx[:, C_DMAT:C_DMAT + CH] = (
            np.arange(P, dtype=np.float32)[:, None]
            - np.arange(CH, dtype=np.float32)[None, :]
        ).astype(np.float16)
        for slot in range(NSLOT):
            cid = CHUNKS_H[h][slot]
            for j in range(NK[slot]):
                cmx[:, C_AMAT + SLOTBASE[slot] + j] = CH * cid - P * j
        cmx[:, C_ONEK] = 1.0
        cmx[0, C_ONER:C_ONER + P] = 1.0
        cmat_h.append(cmx)

    NKC = 4 if USE_GATHER else 8
    in_maps = []
    for core in range(8):
        b, h = core // 2, core % 2
        xq_cols = np.concatenate(
            [np.arange(c * CH, (c + 1) * CH) for c in CHUNKS_H[h]]
        )
        xqT_b = tile_x(np.ascontiguousarray(xT[b][:, xq_cols]), NSLOT, pj_np)
        if USE_GATHER:
            kv_cols = np.arange(4 * h * CH, 4 * (h + 1) * CH)
            xkv = np.ascontiguousarray(xT[b][:, kv_cols])
        else:
            xkv = xT[b]
        xk_t = tile_x(xkv, NKC, pj_np)  # [chunk, P, 8, CH]
        # -> [chunk, dp_pair, P, 2, CH]
        xk_t = np.ascontiguousarray(
            xk_t.reshape(NKC, P, 4, 2, CH).transpose(0, 2, 1, 3, 4)
        )
        in_maps.append(
            {
                "xkT": xk_t,
                "xvT": tile_x(xkv, NKC, np.float16),
                "xqT": xqT_b,
                "wqT": wqT,
                "wkT": wkT,
                "wvT": wvT,
                "cmat": cmat_h[h],
            }
        )
    return in_maps


def _assemble(results):
    out = np.empty((B, S, D), np.float32)
    for core in range(8):
        b, h = core // 2, core % 2
        oT = np.asarray(results[core]["outT"])  # [D, NQ]
        for slot, c in enumerate(CHUNKS_H[h]):
            out[b, c * CH : (c + 1) * CH, :] = oT[:, slot * CH : (slot + 1) * CH].T
    return out


def run(inputs, trace=False, trace_cores=None):
    """Run the kernel; returns (output, BassKernelResults)."""
    from concourse.bass_utils import run_bass_kernel_spmd

    nc = _get_program()
    in_maps = _make_in_maps(
        inputs["x"], inputs["W_query"], inputs["W_key"], inputs["W_value"]
    )
    kw = {}
    if trace:
        kw = dict(trace=True, trace_cores=trace_cores, stitch_traces=False)
    res = run_bass_kernel_spmd(nc, in_maps, list(range(8)), **kw)
    return _assemble(res.results), res


def kernel(x, W_query, W_key, W_value):
    out, _ = run({"x": x, "W_query": W_query, "W_key": W_key, "W_value": W_value})
    return out
